# revision 15
# baseline (speedup 1.0000x reference)
"""Trainium2 Bass kernel for nn_AttentionBlock: 8-core data-parallel over batch.

Reference computation (per batch b):
  cx = X[b] @ Wx^T               [K,R]   (K=49 regions, R=49, H=1024)
  ch = h_t[b] @ Wh^T             [T,R]   (T=128)
  z[t,k] = sum_r Wa[r] * tanh(cx[k,r] + ch[t,r])
  alpha = softmax_k(z)           [T,K]
  out[b] = alpha @ X[b]          [T,H]

v3 design (per core, 16 batches):
  - k-slot mapping: k = 28*J + s (J in {0,1}); zT row(k) = 64*J + s.
    X rows live pre-scattered in xb_all at rows 64*J + s (gaps zeroed).
  - h: SWDGE cast-DMA f32->bf16 (4-batch groups), then 32 SBUF->SBUF xbar
    DMA transposes -> hT chunks. No PE transposes, no DVE casts.
  - ch (mirrored to partitions 0:49 & 64:113 by [WhT|0|WhT] stationary) via
    4-batch quad matmuls; cx for ALL batches upfront via xT_all (128 xbar
    transposes of xb) + 16 matmuls -> cxT2_all bf16 SBUF.
  - S = tanh(ch + cx): one DVE TT (PSUM-src, broadcast APs) split with
    GpSimd TT (SBUF-src) by q-slices; ScalarE tanh.
  - z: 28 col-tiled accumulating matmuls, Wa slabs at col offsets 2rr;
    zT lands [64J+s, t] in one PSUM bank; gap rows written zero.
  - softmax: no max-shift (|z| < 1 by construction); exp PSUM->SBUF bf16 =
    unnormalized alphaT; denom via ones-column matmul; reciprocal;
    normalization fused into the PSUM->SBUF output copy (tensor_scalar /
    activation-Copy with per-partition scale), split DVE/ScalarE.
"""

import sys

sys.path.insert(0, "/opt/trn_rl_repo")

import numpy as np

import concourse.bass as bass
import concourse.bacc as bacc
import concourse.tile as tile
from concourse import mybir
from concourse.bass_utils import run_bass_kernel_spmd
from concourse.masks import make_identity

B, T, K, H = 128, 128, 49, 1024
R = 49
NCORES = 8
BL = B // NCORES  # batches per core
GB = 4  # batches per group
NG = BL // GB
HT = H // 128
NQ = 28  # q slots (14 per J-half)
NQV = 25  # valid q slots
KR = 92  # rows of the scattered k layout (64 + 28)
QD = 13  # q slots added on DVE; rest (NQV-QD) on GpSimd
OSPLIT = 850  # out cols normalized on DVE; rest on ScalarE
F32 = mybir.dt.float32
BF16 = mybir.dt.bfloat16

_CACHE = {}


def _ap(base, off, dims):
    return bass.AP(tensor=base.tensor, offset=base.offset + off, ap=dims)


def build():
    nc = bacc.Bacc("TRN2", target_bir_lowering=False, debug=False, num_devices=NCORES)

    X_d = nc.dram_tensor("X", [BL, K, H], F32, kind="ExternalInput").ap()
    ht_d = nc.dram_tensor("h_t", [BL, T, H], F32, kind="ExternalInput").ap()
    Wx_d = nc.dram_tensor("Wx", [R, H], F32, kind="ExternalInput").ap()
    Wh_d = nc.dram_tensor("Wh", [R, H], F32, kind="ExternalInput").ap()
    Wa_d = nc.dram_tensor("Wa", [1, R], F32, kind="ExternalInput").ap()
    out_d = nc.dram_tensor("out", [BL, T, H], F32, kind="ExternalOutput").ap()

    with tile.TileContext(nc) as tc:
        with (
            tc.tile_pool(name="consts", bufs=1) as consts,
            tc.tile_pool(name="xall", bufs=1) as xall,
            tc.tile_pool(name="hbp", bufs=2) as hb_pool,
            tc.tile_pool(name="hTp", bufs=2) as hT_pool,
            tc.tile_pool(name="chp", bufs=2) as ch_pool,
            tc.tile_pool(name="sp", bufs=3) as s_pool,
            tc.tile_pool(name="ap", bufs=3) as a_pool,
            tc.tile_pool(name="rp", bufs=3) as r_pool,
            tc.tile_pool(name="ob", bufs=3) as o_pool,
            tc.tile_pool(name="pcc", bufs=2, space="PSUM") as pcc,
            tc.tile_pool(name="psZ", bufs=2, space="PSUM") as psZ,
            tc.tile_pool(name="psO", bufs=1, space="PSUM") as psO,
            tc.tile_pool(name="pset", bufs=1, space="PSUM") as pset,
        ):
            # ================= setup =================
            ident = consts.tile([128, 128], F32)
            make_identity(nc, ident[:])

            # weights: natural f32 load, PE transpose, mirrored bf16 copies
            wn = consts.tile([R, 2 * H], F32)
            nc.sync.dma_start(out=wn[:, 0:H], in_=_ap(Wh_d, 0, [[H, R], [1, H]]))
            nc.sync.dma_start(out=wn[:, H : 2 * H], in_=_ap(Wx_d, 0, [[H, R], [1, H]]))
            WhT2 = consts.tile([128, HT, 128], BF16)  # [p, j, 0:49|pad|64:113]
            WxT = consts.tile([128, HT, 64], BF16)
            nc.vector.memset(WhT2[:], 0.0)
            nc.vector.memset(WxT[:], 0.0)
            stile = pset.tile([128, 448], F32, tag="setup")
            for j in range(HT):
                tp = stile[:, j * 56 : j * 56 + R]
                nc.tensor.transpose(
                    tp, wn[:, j * 128 : (j + 1) * 128], ident[0:R, 0:R]
                )
                nc.vector.tensor_copy(WhT2[:, j, 0:R], tp)
                nc.vector.tensor_copy(WhT2[:, j, 64 : 64 + R], tp)
            for j in range(HT):
                tp = stile[:, j * 56 : j * 56 + R]
                nc.tensor.transpose(
                    tp, wn[:, H + j * 128 : H + (j + 1) * 128], ident[0:R, 0:R]
                )
                nc.vector.tensor_copy(WxT[:, j, 0:R], tp)

            # Wa onto partitions, bf16
            waf = consts.tile([R, 1], F32)
            nc.sync.dma_start(out=waf[:], in_=_ap(Wa_d, 0, [[1, R], [1, 1]]))
            wab = consts.tile([R, 1], BF16)
            nc.vector.tensor_copy(wab[:], waf[:])
            # z slabs: [113, 14, 64]; col 2rr <- Wa at rows 0:49, col 2rr+1 <- rows 64:113
            slab = consts.tile([128, 14, 64], BF16)
            nc.vector.memset(slab[:], 0.0)
            for rr in range(14):
                nc.vector.tensor_copy(slab[0:R, rr, 2 * rr : 2 * rr + 1], wab[:])
                nc.vector.tensor_copy(
                    slab[64 : 64 + R, rr, 2 * rr + 1 : 2 * rr + 2], wab[:]
                )

            # ones column for the denominator matmul (valid k rows only)
            onescol = consts.tile([KR, 1], BF16)
            nc.vector.memset(onescol[:], 0.0)
            nc.vector.memset(onescol[0:28, :], 1.0)
            nc.vector.memset(onescol[64 : 64 + (K - 28), :], 1.0)

            # X for all batches: scattered rows 0:28 (k<28) and 64:85 (k>=28)
            xb_all = xall.tile([96, BL, H], BF16)
            nc.gpsimd.memset(xb_all[:], 0.0)
            nc.gpsimd.dma_start(
                out=xb_all[0:28, :, :],
                in_=_ap(X_d, 0, [[H, 28], [K * H, BL], [1, H]]),
            )
            nc.gpsimd.dma_start(
                out=xb_all[64 : 64 + (K - 28), :, :],
                in_=_ap(X_d, 28 * H, [[H, K - 28], [K * H, BL], [1, H]]),
            )
            def _resh(apobj, dims):
                return bass.AP(
                    tensor=apobj.tensor, offset=apobj.offset, ap=[apobj.ap[0]] + dims
                )

            # ================= main loop =================
            def emit_group_fetch(g):
                hb = hb_pool.tile([128, GB, H], BF16, tag="hb")
                nc.gpsimd.dma_start(
                    out=hb[:],
                    in_=_ap(ht_d, g * GB * T * H, [[H, T], [T * H, GB], [1, H]]),
                )
                hTt = hT_pool.tile([128, GB, HT, 128], BF16, tag="hT")
                nc.sync.dma_start(out=hTt[:], in_=hb[:], transpose=True)
                xTg = hT_pool.tile([128, GB, HT, 96], BF16, tag="xT")
                nc.sync.dma_start(
                    out=xTg[:],
                    in_=xb_all[0:96, g * GB : (g + 1) * GB, :],
                    transpose=True,
                )
                return hTt, xTg

            def emit_group_pre(g, hTt, xTg):
                # cx: cxg[p, b', q]; rows 0:49 even-k, 64:113 odd-k
                cxps = pset.tile([128, 448], F32, tag="setup")
                if g < 1:
                    nc.vector.memset(cxps[:], 0.0)
                xt = xTg[:]
                for par in range(2):
                    dst = _resh(
                        cxps[64 * par : 64 * par + R, :], [[NQ, GB], [1, NQ]]
                    )
                    for j in range(HT):
                        nc.tensor.matmul(
                            dst,
                            WxT[:, j, 0:R],
                            _ap(
                                xt,
                                j * 96 + par,
                                [xt.ap[0], [HT * 96, GB], [64, 2], [2, 14]],
                            ),
                            start=(j == 0),
                            stop=(j == HT - 1),
                            tile_position=(0, 64 * par),
                        )
                cxg = ch_pool.tile([128, GB, NQ], BF16, tag="cxg")
                nc.vector.tensor_copy(
                    cxg[0:113, :, :], _resh(cxps[0:113, :], [[NQ, GB], [1, NQ]])
                )
                # ch for the group, mirrored: cc1[p, b', t]
                cc1 = pcc.tile([113, GB, 128], F32, tag="cc1")
                for j in range(HT):
                    nc.tensor.matmul(
                        cc1[:],
                        WhT2[:, j, 0:113],
                        hTt[:, :, j, :],
                        start=(j == 0),
                        stop=(j == HT - 1),
                    )
                chsb = ch_pool.tile([113, GB, 128], BF16, tag="chsb")
                nc.vector.tensor_copy(chsb[:], cc1[:])
                return cxg, cc1, chsb

            def emit_early(b, bb, cxg, cc1, chsb):
                # S = tanh(ch + cx), bf16 [113, 25, 128]
                S = s_pool.tile([128, NQ, 128], BF16, tag="S")
                if b < 3:
                    nc.vector.memset(S[:, NQV:NQ, :], 0.0)
                c1 = cc1[:]
                ca = cxg[:]
                cs = chsb[:]
                nc.vector.tensor_add(
                    S[0:113, 0:QD, :],
                    _ap(c1, bb * 128, [[c1.ap[0][0], 113], [0, QD], [1, 128]]),
                    _ap(ca, bb * NQ, [[ca.ap[0][0], 113], [1, QD], [0, 128]]),
                )
                nc.gpsimd.tensor_tensor(
                    S[0:113, QD:NQV, :],
                    _ap(
                        cs, bb * 128, [[cs.ap[0][0], 113], [0, NQV - QD], [1, 128]]
                    ),
                    _ap(
                        ca,
                        bb * NQ + QD,
                        [[ca.ap[0][0], 113], [1, NQV - QD], [0, 128]],
                    ),
                    mybir.AluOpType.add,
                )
                nc.scalar.activation(
                    S[0:113, 0:NQV, :],
                    S[0:113, 0:NQV, :],
                    mybir.ActivationFunctionType.Tanh,
                )
                # zT[64J+s, t] via col-tiled accumulating matmuls
                zal = psZ.tile([128, 132], F32, tag="z")
                for rr in range(14):
                    for J in range(2):
                        nc.tensor.matmul(
                            zal[64 * J : 64 * J + 64, 0:128],
                            slab[0:113, rr, :],
                            S[0:113, 14 * J + rr, :],
                            start=(rr == 0),
                            stop=(rr == 13),
                            tile_position=(0, 64 * J),
                        )
                return zal

            def emit_late1(b, zal):
                alphaT = a_pool.tile([KR, 128], BF16, tag="alphaT")
                nc.scalar.activation(
                    alphaT[:], zal[0:KR, 0:128], mybir.ActivationFunctionType.Exp
                )
                dps = zal[:, 128:129]
                nc.tensor.matmul(dps, alphaT[:], onescol[:], start=True, stop=True)
                rden = r_pool.tile([128, 1], F32, tag="rden")
                nc.vector.reciprocal(rden[:], dps)
                ob = psO.tile([128, H], F32, tag="ob")
                for half in range(2):
                    nc.tensor.matmul(
                        ob[:, half * 512 : (half + 1) * 512],
                        alphaT[:],
                        xb_all[0:KR, b, half * 512 : (half + 1) * 512],
                        start=True,
                        stop=True,
                    )
                return rden, ob

            def emit_late2(b, rden, ob):
                osb = o_pool.tile([128, H], F32, tag="osb")
                nc.vector.tensor_scalar(
                    osb[:, 0:OSPLIT],
                    ob[:, 0:OSPLIT],
                    rden[:],
                    None,
                    mybir.AluOpType.mult,
                )
                nc.scalar.activation(
                    osb[:, OSPLIT:H],
                    ob[:, OSPLIT:H],
                    mybir.ActivationFunctionType.Copy,
                    scale=rden[:],
                )
                nc.sync.dma_start(
                    out=_ap(out_d, b * T * H, [[H, T], [1, H]]), in_=osb[:]
                )

            fetch = emit_group_fetch(0)
            p1 = None  # (b, zal) awaiting late1
            p2 = None  # (b, rden, ob) awaiting late2
            for g in range(NG):
                hTt, xTg = fetch
                if g + 1 < NG:
                    fetch = emit_group_fetch(g + 1)
                cxg, cc1, chsb = emit_group_pre(g, hTt, xTg)
                for bb in range(GB):
                    b = g * GB + bb
                    if p2 is not None:
                        emit_late2(*p2)
                        p2 = None
                    if p1 is not None:
                        pb, pz = p1
                        rden, ob = emit_late1(pb, pz)
                        p2 = (pb, rden, ob)
                        p1 = None
                    zal = emit_early(b, bb, cxg, cc1, chsb)
                    p1 = (b, zal)
            pb, pz = p1
            if p2 is not None:
                emit_late2(*p2)
            rden, ob = emit_late1(pb, pz)
            emit_late2(pb, rden, ob)

    nc.compile()
    return nc


def _get_nc():
    if "nc" not in _CACHE:
        _CACHE["nc"] = build()
    return _CACHE["nc"]


def kernel(X, h_t, Wx, Wh, Wa):
    nc = _get_nc()
    X = np.ascontiguousarray(X, dtype=np.float32)
    h_t = np.ascontiguousarray(h_t, dtype=np.float32)
    Wx = np.ascontiguousarray(Wx, dtype=np.float32)
    Wh = np.ascontiguousarray(Wh, dtype=np.float32)
    Wa = np.ascontiguousarray(Wa, dtype=np.float32)
    in_maps = [
        {
            "X": X[c * BL : (c + 1) * BL],
            "h_t": h_t[c * BL : (c + 1) * BL],
            "Wx": Wx,
            "Wh": Wh,
            "Wa": Wa,
        }
        for c in range(NCORES)
    ]
    res = run_bass_kernel_spmd(nc, in_maps, core_ids=list(range(NCORES)))
    return np.concatenate([res.results[c]["out"] for c in range(NCORES)], axis=0)


# revision 16
# speedup vs baseline: 1.0318x; 1.0318x over previous
"""Trainium2 Bass kernel for nn_AttentionBlock: 8-core data-parallel over batch.

Reference computation (per batch b):
  cx = X[b] @ Wx^T               [K,R]   (K=49 regions, R=49, H=1024)
  ch = h_t[b] @ Wh^T             [T,R]   (T=128)
  z[t,k] = sum_r Wa[r] * tanh(cx[k,r] + ch[t,r])
  alpha = softmax_k(z)           [T,K]
  out[b] = alpha @ X[b]          [T,H]

v3 design (per core, 16 batches):
  - k-slot mapping: k = 28*J + s (J in {0,1}); zT row(k) = 64*J + s.
    X rows live pre-scattered in xb_all at rows 64*J + s (gaps zeroed).
  - h: SWDGE cast-DMA f32->bf16 (4-batch groups), then 32 SBUF->SBUF xbar
    DMA transposes -> hT chunks. No PE transposes, no DVE casts.
  - ch (mirrored to partitions 0:49 & 64:113 by [WhT|0|WhT] stationary) via
    4-batch quad matmuls; cx for ALL batches upfront via xT_all (128 xbar
    transposes of xb) + 16 matmuls -> cxT2_all bf16 SBUF.
  - S = tanh(ch + cx): one DVE TT (PSUM-src, broadcast APs) split with
    GpSimd TT (SBUF-src) by q-slices; ScalarE tanh.
  - z: 28 col-tiled accumulating matmuls, Wa slabs at col offsets 2rr;
    zT lands [64J+s, t] in one PSUM bank; gap rows written zero.
  - softmax: no max-shift (|z| < 1 by construction); exp PSUM->SBUF bf16 =
    unnormalized alphaT; denom via ones-column matmul; reciprocal;
    normalization fused into the PSUM->SBUF output copy (tensor_scalar /
    activation-Copy with per-partition scale), split DVE/ScalarE.
"""

import sys

sys.path.insert(0, "/opt/trn_rl_repo")

import numpy as np

import concourse.bass as bass
import concourse.bacc as bacc
import concourse.tile as tile
from concourse import mybir
from concourse.bass_utils import run_bass_kernel_spmd
from concourse.masks import make_identity

B, T, K, H = 128, 128, 49, 1024
R = 49
NCORES = 8
BL = B // NCORES  # batches per core
GB = 4  # batches per group
NG = BL // GB
HT = H // 128
NQ = 28  # q slots (14 per J-half)
NQV = 25  # valid q slots
KR = 92  # rows of the scattered k layout (64 + 28)
QD = 13  # q slots added on DVE; rest (NQV-QD) on GpSimd
OSPLIT = 850  # out cols normalized on DVE; rest on ScalarE
F32 = mybir.dt.float32
BF16 = mybir.dt.bfloat16

_CACHE = {}


def _ap(base, off, dims):
    return bass.AP(tensor=base.tensor, offset=base.offset + off, ap=dims)


def build():
    nc = bacc.Bacc("TRN2", target_bir_lowering=False, debug=False, num_devices=NCORES)

    X_d = nc.dram_tensor("X", [BL, K, H], F32, kind="ExternalInput").ap()
    ht_d = nc.dram_tensor("h_t", [BL, T, H], F32, kind="ExternalInput").ap()
    Wx_d = nc.dram_tensor("Wx", [R, H], F32, kind="ExternalInput").ap()
    Wh_d = nc.dram_tensor("Wh", [R, H], F32, kind="ExternalInput").ap()
    Wa_d = nc.dram_tensor("Wa", [1, R], F32, kind="ExternalInput").ap()
    out_d = nc.dram_tensor("out", [BL, T, H], F32, kind="ExternalOutput").ap()

    with tile.TileContext(nc) as tc:
        with (
            tc.tile_pool(name="consts", bufs=1) as consts,
            tc.tile_pool(name="xall", bufs=1) as xall,
            tc.tile_pool(name="hbp", bufs=2) as hb_pool,
            tc.tile_pool(name="hTp", bufs=2) as hT_pool,
            tc.tile_pool(name="chp", bufs=2) as ch_pool,
            tc.tile_pool(name="sp", bufs=3) as s_pool,
            tc.tile_pool(name="ap", bufs=3) as a_pool,
            tc.tile_pool(name="rp", bufs=3) as r_pool,
            tc.tile_pool(name="ob", bufs=3) as o_pool,
            tc.tile_pool(name="pcc", bufs=2, space="PSUM") as pcc,
            tc.tile_pool(name="psZ", bufs=2, space="PSUM") as psZ,
            tc.tile_pool(name="psO", bufs=1, space="PSUM") as psO,
            tc.tile_pool(name="pset", bufs=1, space="PSUM") as pset,
        ):
            # ================= setup =================
            # X tile first: gap rows must be zero; split the big memset
            xb_all = xall.tile([96, BL, H], BF16)
            nc.vector.memset(xb_all[:, 0 : BL // 2, :], 0.0)
            nc.gpsimd.memset(xb_all[:, BL // 2 : BL, :], 0.0)

            ident = consts.tile([128, 128], F32)
            make_identity(nc, ident[:])

            # weights: natural f32 load, PE transpose, mirrored bf16 copies.
            # Wh transposes rotate through the psZ pool (2 bufs), Wx through pset.
            wn = consts.tile([R, 2 * H], F32)
            nc.sync.dma_start(out=wn[:, 0:H], in_=_ap(Wh_d, 0, [[H, R], [1, H]]))
            nc.sync.dma_start(out=wn[:, H : 2 * H], in_=_ap(Wx_d, 0, [[H, R], [1, H]]))
            WhT2 = consts.tile([128, HT, 128], BF16)  # [p, j, 0:49|pad|64:113]
            WxT = consts.tile([128, HT, 64], BF16)
            nc.vector.memset(WhT2[:], 0.0)
            nc.vector.memset(WxT[:], 0.0)
            stile = pset.tile([128, 448], F32, tag="setup")
            for j in range(HT):
                zt = psZ.tile([128, 132], F32, tag="z")
                tph = zt[:, 0:R]
                nc.tensor.transpose(
                    tph, wn[:, j * 128 : (j + 1) * 128], ident[0:R, 0:R]
                )
                tpx = stile[:, j * 56 : j * 56 + R]
                nc.tensor.transpose(
                    tpx, wn[:, H + j * 128 : H + (j + 1) * 128], ident[0:R, 0:R]
                )
                nc.vector.tensor_copy(WxT[:, j, 0:R], tpx)
                nc.vector.tensor_copy(WhT2[:, j, 0:R], tph)
                nc.vector.tensor_copy(WhT2[:, j, 64 : 64 + R], tph)

            # Wa onto partitions, bf16
            waf = consts.tile([R, 1], F32)
            nc.sync.dma_start(out=waf[:], in_=_ap(Wa_d, 0, [[1, R], [1, 1]]))
            wab = consts.tile([R, 1], BF16)
            nc.vector.tensor_copy(wab[:], waf[:])
            # z slabs: [113, 14, 64]; col 2rr <- Wa at rows 0:49, col 2rr+1 <- rows 64:113
            slab = consts.tile([128, 14, 64], BF16)
            nc.vector.memset(slab[:], 0.0)
            for rr in range(14):
                nc.vector.tensor_copy(slab[0:R, rr, 2 * rr : 2 * rr + 1], wab[:])
                nc.vector.tensor_copy(
                    slab[64 : 64 + R, rr, 2 * rr + 1 : 2 * rr + 2], wab[:]
                )

            # ones column for the denominator matmul (valid k rows only)
            onescol = consts.tile([KR, 1], BF16)
            nc.vector.memset(onescol[:], 0.0)
            nc.vector.memset(onescol[0:28, :], 1.0)
            nc.vector.memset(onescol[64 : 64 + (K - 28), :], 1.0)

            # X for all batches: scattered rows 0:28 (k<28) and 64:85 (k>=28)
            nc.gpsimd.dma_start(
                out=xb_all[0:28, :, :],
                in_=_ap(X_d, 0, [[H, 28], [K * H, BL], [1, H]]),
            )
            nc.gpsimd.dma_start(
                out=xb_all[64 : 64 + (K - 28), :, :],
                in_=_ap(X_d, 28 * H, [[H, K - 28], [K * H, BL], [1, H]]),
            )
            def _resh(apobj, dims):
                return bass.AP(
                    tensor=apobj.tensor, offset=apobj.offset, ap=[apobj.ap[0]] + dims
                )

            # ================= main loop =================
            def emit_group_fetch(g):
                hb = hb_pool.tile([128, GB, H], BF16, tag="hb")
                nc.gpsimd.dma_start(
                    out=hb[:],
                    in_=_ap(ht_d, g * GB * T * H, [[H, T], [T * H, GB], [1, H]]),
                )
                hTt = hT_pool.tile([128, GB, HT, 128], BF16, tag="hT")
                nc.sync.dma_start(out=hTt[:], in_=hb[:], transpose=True)
                xTg = hT_pool.tile([128, GB, HT, 96], BF16, tag="xT")
                nc.sync.dma_start(
                    out=xTg[:],
                    in_=xb_all[0:96, g * GB : (g + 1) * GB, :],
                    transpose=True,
                )
                return hTt, xTg

            def emit_group_pre(g, hTt, xTg):
                # cx: cxg[p, b', q]; rows 0:49 even-k, 64:113 odd-k
                cxps = pset.tile([128, 448], F32, tag="setup")
                if g < 1:
                    nc.vector.memset(cxps[:], 0.0)
                xt = xTg[:]
                for par in range(2):
                    dst = _resh(
                        cxps[64 * par : 64 * par + R, :], [[NQ, GB], [1, NQ]]
                    )
                    for j in range(HT):
                        nc.tensor.matmul(
                            dst,
                            WxT[:, j, 0:R],
                            _ap(
                                xt,
                                j * 96 + par,
                                [xt.ap[0], [HT * 96, GB], [64, 2], [2, 14]],
                            ),
                            start=(j == 0),
                            stop=(j == HT - 1),
                            tile_position=(0, 64 * par),
                        )
                cxg = ch_pool.tile([128, GB, NQ], BF16, tag="cxg")
                nc.vector.tensor_copy(
                    cxg[0:113, :, :], _resh(cxps[0:113, :], [[NQ, GB], [1, NQ]])
                )
                # ch for the group, mirrored: cc1[p, b', t]
                cc1 = pcc.tile([113, GB, 128], F32, tag="cc1")
                for j in range(HT):
                    nc.tensor.matmul(
                        cc1[:],
                        WhT2[:, j, 0:113],
                        hTt[:, :, j, :],
                        start=(j == 0),
                        stop=(j == HT - 1),
                    )
                chsb = ch_pool.tile([113, GB, 128], BF16, tag="chsb")
                nc.vector.tensor_copy(chsb[:], cc1[:])
                return cxg, cc1, chsb

            def emit_early(b, bb, cxg, cc1, chsb):
                # S = tanh(ch + cx), bf16 [113, 25, 128]
                S = s_pool.tile([128, NQ, 128], BF16, tag="S")
                if b < 3:
                    nc.vector.memset(S[:, NQV:NQ, :], 0.0)
                c1 = cc1[:]
                ca = cxg[:]
                cs = chsb[:]
                nc.vector.tensor_add(
                    S[0:113, 0:QD, :],
                    _ap(c1, bb * 128, [[c1.ap[0][0], 113], [0, QD], [1, 128]]),
                    _ap(ca, bb * NQ, [[ca.ap[0][0], 113], [1, QD], [0, 128]]),
                )
                nc.gpsimd.tensor_tensor(
                    S[0:113, QD:NQV, :],
                    _ap(
                        cs, bb * 128, [[cs.ap[0][0], 113], [0, NQV - QD], [1, 128]]
                    ),
                    _ap(
                        ca,
                        bb * NQ + QD,
                        [[ca.ap[0][0], 113], [1, NQV - QD], [0, 128]],
                    ),
                    mybir.AluOpType.add,
                )
                nc.scalar.activation(
                    S[0:113, 0:NQV, :],
                    S[0:113, 0:NQV, :],
                    mybir.ActivationFunctionType.Tanh,
                )
                # zT[64J+s, t] via col-tiled accumulating matmuls
                zal = psZ.tile([128, 132], F32, tag="z")
                for rr in range(14):
                    for J in range(2):
                        nc.tensor.matmul(
                            zal[64 * J : 64 * J + 64, 0:128],
                            slab[0:113, rr, :],
                            S[0:113, 14 * J + rr, :],
                            start=(rr == 0),
                            stop=(rr == 13),
                            tile_position=(0, 64 * J),
                        )
                return zal

            def emit_late1(b, zal):
                alphaT = a_pool.tile([KR, 128], BF16, tag="alphaT")
                nc.scalar.activation(
                    alphaT[:], zal[0:KR, 0:128], mybir.ActivationFunctionType.Exp
                )
                dps = zal[:, 128:129]
                nc.tensor.matmul(dps, alphaT[:], onescol[:], start=True, stop=True)
                rden = r_pool.tile([128, 1], F32, tag="rden")
                nc.vector.reciprocal(rden[:], dps)
                ob = psO.tile([128, H], F32, tag="ob")
                for half in range(2):
                    nc.tensor.matmul(
                        ob[:, half * 512 : (half + 1) * 512],
                        alphaT[:],
                        xb_all[0:KR, b, half * 512 : (half + 1) * 512],
                        start=True,
                        stop=True,
                    )
                return rden, ob

            def emit_late2(b, rden, ob):
                osb = o_pool.tile([128, H], F32, tag="osb")
                nc.vector.tensor_scalar(
                    osb[:, 0:OSPLIT],
                    ob[:, 0:OSPLIT],
                    rden[:],
                    None,
                    mybir.AluOpType.mult,
                )
                nc.scalar.activation(
                    osb[:, OSPLIT:H],
                    ob[:, OSPLIT:H],
                    mybir.ActivationFunctionType.Copy,
                    scale=rden[:],
                )
                nc.sync.dma_start(
                    out=_ap(out_d, b * T * H, [[H, T], [1, H]]), in_=osb[:]
                )

            fetch = emit_group_fetch(0)
            p1 = None  # (b, zal) awaiting late1
            p2 = None  # (b, rden, ob) awaiting late2
            for g in range(NG):
                hTt, xTg = fetch
                if g + 1 < NG:
                    fetch = emit_group_fetch(g + 1)
                cxg, cc1, chsb = emit_group_pre(g, hTt, xTg)
                for bb in range(GB):
                    b = g * GB + bb
                    if p2 is not None:
                        emit_late2(*p2)
                        p2 = None
                    if p1 is not None:
                        pb, pz = p1
                        rden, ob = emit_late1(pb, pz)
                        p2 = (pb, rden, ob)
                        p1 = None
                    zal = emit_early(b, bb, cxg, cc1, chsb)
                    p1 = (b, zal)
            pb, pz = p1
            if p2 is not None:
                emit_late2(*p2)
            rden, ob = emit_late1(pb, pz)
            emit_late2(pb, rden, ob)

    nc.compile()
    return nc


def _get_nc():
    if "nc" not in _CACHE:
        _CACHE["nc"] = build()
    return _CACHE["nc"]


def kernel(X, h_t, Wx, Wh, Wa):
    nc = _get_nc()
    X = np.ascontiguousarray(X, dtype=np.float32)
    h_t = np.ascontiguousarray(h_t, dtype=np.float32)
    Wx = np.ascontiguousarray(Wx, dtype=np.float32)
    Wh = np.ascontiguousarray(Wh, dtype=np.float32)
    Wa = np.ascontiguousarray(Wa, dtype=np.float32)
    in_maps = [
        {
            "X": X[c * BL : (c + 1) * BL],
            "h_t": h_t[c * BL : (c + 1) * BL],
            "Wx": Wx,
            "Wh": Wh,
            "Wa": Wa,
        }
        for c in range(NCORES)
    ]
    res = run_bass_kernel_spmd(nc, in_maps, core_ids=list(range(NCORES)))
    return np.concatenate([res.results[c]["out"] for c in range(NCORES)], axis=0)


# revision 17
# speedup vs baseline: 1.1972x; 1.1604x over previous
"""Trainium2 Bass kernel for nn_AttentionBlock: 8-core data-parallel over batch.

Reference computation (per batch b):
  cx = X[b] @ Wx^T               [K,R]   (K=49 regions, R=49, H=1024)
  ch = h_t[b] @ Wh^T             [T,R]   (T=128)
  z[t,k] = sum_r Wa[r] * tanh(cx[k,r] + ch[t,r])
  alpha = softmax_k(z)           [T,K]
  out[b] = alpha @ X[b]          [T,H]

v3 design (per core, 16 batches):
  - k-slot mapping: k = 28*J + s (J in {0,1}); zT row(k) = 64*J + s.
    X rows live pre-scattered in xb_all at rows 64*J + s (gaps zeroed).
  - h: SWDGE cast-DMA f32->bf16 (4-batch groups), then 32 SBUF->SBUF xbar
    DMA transposes -> hT chunks. No PE transposes, no DVE casts.
  - ch (mirrored to partitions 0:49 & 64:113 by [WhT|0|WhT] stationary) via
    4-batch quad matmuls; cx for ALL batches upfront via xT_all (128 xbar
    transposes of xb) + 16 matmuls -> cxT2_all bf16 SBUF.
  - S = tanh(ch + cx): one DVE TT (PSUM-src, broadcast APs) split with
    GpSimd TT (SBUF-src) by q-slices; ScalarE tanh.
  - z: 28 col-tiled accumulating matmuls, Wa slabs at col offsets 2rr;
    zT lands [64J+s, t] in one PSUM bank; gap rows written zero.
  - softmax: no max-shift (|z| < 1 by construction); exp PSUM->SBUF bf16 =
    unnormalized alphaT; denom via ones-column matmul; reciprocal;
    normalization fused into the PSUM->SBUF output copy (tensor_scalar /
    activation-Copy with per-partition scale), split DVE/ScalarE.
"""

import sys

sys.path.insert(0, "/opt/trn_rl_repo")

import numpy as np

import concourse.bass as bass
import concourse.bacc as bacc
import concourse.tile as tile
from concourse import mybir
from concourse.bass_utils import run_bass_kernel_spmd
from concourse.masks import make_identity

B, T, K, H = 128, 128, 49, 1024
R = 49
NCORES = 8
BL = B // NCORES  # batches per core
GB = 4  # batches per group
NG = BL // GB
HT = H // 128
NQ = 28  # q slots (14 per J-half)
NQV = 25  # valid q slots
KR = 92  # rows of the scattered k layout (64 + 28)
QD = 13  # q slots added on DVE; rest (NQV-QD) on GpSimd
OSPLIT = 850  # out cols normalized on DVE; rest on ScalarE
F32 = mybir.dt.float32
BF16 = mybir.dt.bfloat16

_CACHE = {}


def _ap(base, off, dims):
    return bass.AP(tensor=base.tensor, offset=base.offset + off, ap=dims)


def build():
    nc = bacc.Bacc("TRN2", target_bir_lowering=False, debug=False, num_devices=NCORES)

    X_d = nc.dram_tensor("X", [BL, K, H], F32, kind="ExternalInput").ap()
    ht_d = nc.dram_tensor("h_t", [BL, T, H], F32, kind="ExternalInput").ap()
    Wx_d = nc.dram_tensor("Wx", [R, H], F32, kind="ExternalInput").ap()
    Wh_d = nc.dram_tensor("Wh", [R, H], F32, kind="ExternalInput").ap()
    Wa_d = nc.dram_tensor("Wa", [1, R], F32, kind="ExternalInput").ap()
    out_d = nc.dram_tensor("out", [BL, T, H], F32, kind="ExternalOutput").ap()

    with tile.TileContext(nc) as tc:
        with (
            tc.tile_pool(name="consts", bufs=1) as consts,
            tc.tile_pool(name="xall", bufs=1) as xall,
            tc.tile_pool(name="hbp", bufs=2) as hb_pool,
            tc.tile_pool(name="hTp", bufs=2) as hT_pool,
            tc.tile_pool(name="chp", bufs=2) as ch_pool,
            tc.tile_pool(name="sp", bufs=3) as s_pool,
            tc.tile_pool(name="ap", bufs=3) as a_pool,
            tc.tile_pool(name="rp", bufs=3) as r_pool,
            tc.tile_pool(name="ob", bufs=3) as o_pool,
            tc.tile_pool(name="pcc", bufs=2, space="PSUM") as pcc,
            tc.tile_pool(name="psZ", bufs=2, space="PSUM") as psZ,
            tc.tile_pool(name="psO", bufs=1, space="PSUM") as psO,
            tc.tile_pool(name="pset", bufs=1, space="PSUM") as pset,
        ):
            # ================= setup =================
            # X tile first: gap rows must be zero; split the big memset
            xb_all = xall.tile([96, BL, H], BF16)
            nc.vector.memset(xb_all[:, 0 : BL // 2, :], 0.0)
            nc.gpsimd.memset(xb_all[:, BL // 2 : BL, :], 0.0)

            ident = consts.tile([128, 128], F32)
            make_identity(nc, ident[:])

            # weights: natural f32 load, PE transpose, mirrored bf16 copies.
            # Wh transposes rotate through the psZ pool (2 bufs), Wx through pset.
            wn = consts.tile([R, 2 * H], F32)
            nc.sync.dma_start(out=wn[:, 0:H], in_=_ap(Wh_d, 0, [[H, R], [1, H]]))
            nc.sync.dma_start(out=wn[:, H : 2 * H], in_=_ap(Wx_d, 0, [[H, R], [1, H]]))
            WhT2 = consts.tile([128, HT, 128], BF16)  # [p, j, 0:49|pad|64:113]
            WxT = consts.tile([128, HT, 64], BF16)
            nc.vector.memset(WhT2[:], 0.0)
            nc.vector.memset(WxT[:], 0.0)
            stile = pset.tile([128, 448], F32, tag="setup")
            for j in range(HT):
                zt = psZ.tile([128, 132], F32, tag="z")
                tph = zt[:, 0:R]
                nc.tensor.transpose(
                    tph, wn[:, j * 128 : (j + 1) * 128], ident[0:R, 0:R]
                )
                tpx = stile[:, j * 56 : j * 56 + R]
                nc.tensor.transpose(
                    tpx, wn[:, H + j * 128 : H + (j + 1) * 128], ident[0:R, 0:R]
                )
                nc.vector.tensor_copy(WxT[:, j, 0:R], tpx)
                nc.vector.tensor_copy(WhT2[:, j, 0:R], tph)
                nc.vector.tensor_copy(WhT2[:, j, 64 : 64 + R], tph)

            # Wa onto partitions, bf16
            waf = consts.tile([R, 1], F32)
            nc.sync.dma_start(out=waf[:], in_=_ap(Wa_d, 0, [[1, R], [1, 1]]))
            wab = consts.tile([R, 1], BF16)
            nc.vector.tensor_copy(wab[:], waf[:])
            # z slabs: [113, 14, 64]; col 2rr <- Wa at rows 0:49, col 2rr+1 <- rows 64:113
            slab = consts.tile([128, 14, 64], BF16)
            nc.vector.memset(slab[:], 0.0)
            for rr in range(14):
                nc.vector.tensor_copy(slab[0:R, rr, 2 * rr : 2 * rr + 1], wab[:])
                nc.vector.tensor_copy(
                    slab[64 : 64 + R, rr, 2 * rr + 1 : 2 * rr + 2], wab[:]
                )

            # ones column for the denominator matmul (valid k rows only)
            onescol = consts.tile([KR, 1], BF16)
            nc.vector.memset(onescol[:], 0.0)
            nc.vector.memset(onescol[0:28, :], 1.0)
            nc.vector.memset(onescol[64 : 64 + (K - 28), :], 1.0)

            # X for all batches: scattered rows 0:28 (k<28) and 64:85 (k>=28)
            nc.gpsimd.dma_start(
                out=xb_all[0:28, :, :],
                in_=_ap(X_d, 0, [[H, 28], [K * H, BL], [1, H]]),
            )
            nc.gpsimd.dma_start(
                out=xb_all[64 : 64 + (K - 28), :, :],
                in_=_ap(X_d, 28 * H, [[H, K - 28], [K * H, BL], [1, H]]),
            )
            def _resh(apobj, dims):
                return bass.AP(
                    tensor=apobj.tensor, offset=apobj.offset, ap=[apobj.ap[0]] + dims
                )

            # ================= main loop =================
            def emit_group_fetch(g):
                hb = hb_pool.tile([128, GB, H], BF16, tag="hb")
                nc.gpsimd.dma_start(
                    out=hb[:],
                    in_=_ap(ht_d, g * GB * T * H, [[H, T], [T * H, GB], [1, H]]),
                )
                hTt = hT_pool.tile([128, GB, HT, 128], BF16, tag="hT")
                nc.sync.dma_start(out=hTt[:], in_=hb[:], transpose=True)
                xTg = hT_pool.tile([128, GB, HT, 96], BF16, tag="xT")
                nc.sync.dma_start(
                    out=xTg[:],
                    in_=xb_all[0:96, g * GB : (g + 1) * GB, :],
                    transpose=True,
                )
                return hTt, xTg

            def emit_group_pre(g, hTt, xTg):
                # cx: cxg[p, b', q]; rows 0:49 even-k, 64:113 odd-k
                cxps = pset.tile([128, 448], F32, tag="setup")
                if g < 1:
                    nc.vector.memset(cxps[:], 0.0)
                xt = xTg[:]
                for par in range(2):
                    dst = _resh(
                        cxps[64 * par : 64 * par + R, :], [[NQ, GB], [1, NQ]]
                    )
                    for j in range(HT):
                        nc.tensor.matmul(
                            dst,
                            WxT[:, j, 0:R],
                            _ap(
                                xt,
                                j * 96 + par,
                                [xt.ap[0], [HT * 96, GB], [64, 2], [2, 14]],
                            ),
                            start=(j == 0),
                            stop=(j == HT - 1),
                            tile_position=(0, 64 * par),
                        )
                cxg = ch_pool.tile([128, GB, NQ], BF16, tag="cxg")
                nc.vector.tensor_copy(
                    cxg[0:113, :, :], _resh(cxps[0:113, :], [[NQ, GB], [1, NQ]])
                )
                # ch for the group, mirrored: cc1[p, b', t]
                cc1 = pcc.tile([113, GB, 128], F32, tag="cc1")
                for j in range(HT):
                    nc.tensor.matmul(
                        cc1[:],
                        WhT2[:, j, 0:113],
                        hTt[:, :, j, :],
                        start=(j == 0),
                        stop=(j == HT - 1),
                    )
                chsb = ch_pool.tile([113, GB, 128], BF16, tag="chsb")
                nc.vector.tensor_copy(chsb[:], cc1[:])
                return cxg, cc1, chsb

            def emit_early(b, bb, cxg, cc1, chsb):
                # S = tanh(ch + cx), bf16 [113, 25, 128]
                S = s_pool.tile([128, NQ, 128], BF16, tag="S")
                if b < 3:
                    nc.vector.memset(S[:, NQV:NQ, :], 0.0)
                c1 = cc1[:]
                ca = cxg[:]
                cs = chsb[:]
                nc.vector.tensor_add(
                    S[0:113, 0:QD, :],
                    _ap(c1, bb * 128, [[c1.ap[0][0], 113], [0, QD], [1, 128]]),
                    _ap(ca, bb * NQ, [[ca.ap[0][0], 113], [1, QD], [0, 128]]),
                )
                nc.gpsimd.tensor_tensor(
                    S[0:113, QD:NQV, :],
                    _ap(
                        cs, bb * 128, [[cs.ap[0][0], 113], [0, NQV - QD], [1, 128]]
                    ),
                    _ap(
                        ca,
                        bb * NQ + QD,
                        [[ca.ap[0][0], 113], [1, NQV - QD], [0, 128]],
                    ),
                    mybir.AluOpType.add,
                )
                nc.scalar.activation(
                    S[0:113, 0:NQV, :],
                    S[0:113, 0:NQV, :],
                    mybir.ActivationFunctionType.Tanh,
                )
                # zT[64J+s, t] via col-tiled accumulating matmuls
                zal = psZ.tile([128, 132], F32, tag="z")
                for rr in range(14):
                    for J in range(2):
                        nc.tensor.matmul(
                            zal[64 * J : 64 * J + 64, 0:128],
                            slab[0:113, rr, :],
                            S[0:113, 14 * J + rr, :],
                            start=(rr == 0),
                            stop=(rr == 13),
                            tile_position=(0, 64 * J),
                        )
                return zal

            def emit_late1(b, zal):
                alphaT = a_pool.tile([KR, 128], BF16, tag="alphaT")
                nc.scalar.activation(
                    alphaT[:], zal[0:KR, 0:128], mybir.ActivationFunctionType.Exp
                )
                dps = zal[:, 128:129]
                nc.tensor.matmul(dps, alphaT[:], onescol[:], start=True, stop=True)
                rden = r_pool.tile([128, 1], F32, tag="rden")
                nc.vector.reciprocal(rden[:], dps)
                ob = psO.tile([128, H], F32, tag="ob")
                for half in range(2):
                    nc.tensor.matmul(
                        ob[:, half * 512 : (half + 1) * 512],
                        alphaT[:],
                        xb_all[0:KR, b, half * 512 : (half + 1) * 512],
                        start=True,
                        stop=True,
                    )
                return rden, ob

            def emit_late2(b, rden, ob):
                osb = o_pool.tile([128, H], F32, tag="osb")
                nc.vector.tensor_scalar(
                    osb[:, 0:OSPLIT],
                    ob[:, 0:OSPLIT],
                    rden[:],
                    None,
                    mybir.AluOpType.mult,
                )
                nc.scalar.activation(
                    osb[:, OSPLIT:H],
                    ob[:, OSPLIT:H],
                    mybir.ActivationFunctionType.Copy,
                    scale=rden[:],
                )
                nc.sync.dma_start(
                    out=_ap(out_d, b * T * H, [[H, T], [1, H]]), in_=osb[:]
                )

            fetch = emit_group_fetch(0)
            for g in range(NG):
                hTt, xTg = fetch
                if g + 1 < NG:
                    fetch = emit_group_fetch(g + 1)
                cxg, cc1, chsb = emit_group_pre(g, hTt, xTg)
                for bb in range(GB):
                    b = g * GB + bb
                    zal = emit_early(b, bb, cxg, cc1, chsb)
                    rden, ob = emit_late1(b, zal)
                    emit_late2(b, rden, ob)

    nc.compile()
    return nc


def _get_nc():
    if "nc" not in _CACHE:
        _CACHE["nc"] = build()
    return _CACHE["nc"]


def kernel(X, h_t, Wx, Wh, Wa):
    nc = _get_nc()
    X = np.ascontiguousarray(X, dtype=np.float32)
    h_t = np.ascontiguousarray(h_t, dtype=np.float32)
    Wx = np.ascontiguousarray(Wx, dtype=np.float32)
    Wh = np.ascontiguousarray(Wh, dtype=np.float32)
    Wa = np.ascontiguousarray(Wa, dtype=np.float32)
    in_maps = [
        {
            "X": X[c * BL : (c + 1) * BL],
            "h_t": h_t[c * BL : (c + 1) * BL],
            "Wx": Wx,
            "Wh": Wh,
            "Wa": Wa,
        }
        for c in range(NCORES)
    ]
    res = run_bass_kernel_spmd(nc, in_maps, core_ids=list(range(NCORES)))
    return np.concatenate([res.results[c]["out"] for c in range(NCORES)], axis=0)
